# revision 61
# baseline (speedup 1.0000x reference)
"""Trainium2 Bass kernel for a ConvViT-style dense transformer block.

Reference computation (B=2, N=3136=56x56, C=512, 8 heads, hidden 2048):
    x = x + Attn(LN1(x));  x = x + MLP(LN2(x))
    MLP = fc2(gelu(dwconv3x3(fc1(.)) + dw_b))

Sharding: tokens are sharded 8 ways as (batch, 14-image-row) stripes.
Each core computes attention/MLP for its own 14 rows (plus 1 halo row on
each side for the depthwise conv), recomputing K/V projections for its
full batch locally (no collectives).  Host does the (free) scatter/gather.

v2: exp batched per kt (one ACTIVATE over both head-quadrants), attention
starts early (QT/KT emitted first, V5 interleaved with the first head
pair), transposes as matmul-by-identity, bias matmuls folded away, the
3x3 depthwise conv runs 6 taps on PE (diagonal stationary) + 3 on DVE.
"""

import numpy as np

# ---------------- problem constants (hardcoded per spec) ----------------
B = 2
HI = 56          # image rows
WI = 56          # image cols
NB = HI * WI     # tokens per batch = 3136
C = 512
NH = 8
HD = 64
F3 = 3 * C       # 1536
HID = 4 * C      # 2048
EPS = 1e-5
NCORES = 8
RPC = HI // 4    # image rows per core = 14
EXTR = RPC + 2   # rows incl halo = 16
EXT = EXTR * WI  # 896 ext tokens
OWN = RPC * WI   # 784 own tokens
QCH = EXT // 2   # 448 q-chunk
PE_TAPS = (0, 1, 2, 3, 5, 6, 8)   # conv taps on PE (diag matmul)
DVE_TAPS = (4, 7)                 # conv taps on DVE

_CACHE = {}


def _btiles():
    # 128-token tiles over the full batch (24 x 128 + 1 x 64)
    return [(i * 128, min(128, NB - i * 128)) for i in range((NB + 127) // 128)]


def _bchunks():
    # 512-token chunks over the full batch (6 x 512 + 1 x 64)
    return [(i * 512, min(512, NB - i * 512)) for i in range((NB + 511) // 512)]


def _build_nc():
    import concourse.bass as bass
    import concourse.bacc as bacc
    import concourse.tile as tile
    from concourse import mybir

    f32 = mybir.dt.float32
    b16 = mybir.dt.bfloat16
    AF = mybir.ActivationFunctionType
    OP = mybir.AluOpType

    nc = bacc.Bacc(trn_type="TRN2")

    # ---- external I/O ----
    xb_d = nc.dram_tensor("xb", [NB, C], f32, kind="ExternalInput")
    xe_d = nc.dram_tensor("xe", [EXT, C], f32, kind="ExternalInput")
    mask_d = nc.dram_tensor("mask", [EXT], b16, kind="ExternalInput")
    qkvT_d = nc.dram_tensor("qkvT", [C, F3], b16, kind="ExternalInput")
    qb_d = nc.dram_tensor("qb", [128, 4], f32, kind="ExternalInput")
    outT_d = nc.dram_tensor("outT", [C, C], b16, kind="ExternalInput")
    outb_d = nc.dram_tensor("outb", [1, C], b16, kind="ExternalInput")
    fc1T_d = nc.dram_tensor("fc1T", [C, HID], b16, kind="ExternalInput")
    fc1bp_d = nc.dram_tensor("fc1bp", [128, 16], f32, kind="ExternalInput")
    fc2T_d = nc.dram_tensor("fc2T", [HID, C], b16, kind="ExternalInput")
    fc2b_d = nc.dram_tensor("fc2b", [1, C], b16, kind="ExternalInput")
    dww_d = nc.dram_tensor("dww", [HID, 9], f32, kind="ExternalInput")
    dwb_d = nc.dram_tensor("dwb", [HID], f32, kind="ExternalInput")
    dwdiag_d = nc.dram_tensor("dwdiag", [16, 128, len(PE_TAPS) * 128], b16,
                              kind="ExternalInput")
    ident_d = nc.dram_tensor("ident", [128, 128], b16, kind="ExternalInput")
    out_d = nc.dram_tensor("out", [OWN, C], f32, kind="ExternalOutput")

    btiles = _btiles()
    bchunks = _bchunks()
    etiles = [(i * 128, 128) for i in range(EXT // 128)]          # 7 x 128
    otiles = [(i * 128, min(128, OWN - i * 128)) for i in range((OWN + 127) // 128)]

    with tile.TileContext(nc) as tc:
        from contextlib import ExitStack

        with ExitStack() as ctx:
            wp = ctx.enter_context(tc.tile_pool(name="wp", bufs=1))
            big = ctx.enter_context(tc.tile_pool(name="big", bufs=1))
            stage = ctx.enter_context(tc.tile_pool(name="stage", bufs=6))
            small = ctx.enter_context(tc.tile_pool(name="small", bufs=8))
            atp = ctx.enter_context(tc.tile_pool(name="atp", bufs=4))
            padp = ctx.enter_context(tc.tile_pool(name="padp", bufs=2))
            dgp = ctx.enter_context(tc.tile_pool(name="dgp", bufs=2))
            # PSUM: tr(1 bank x2) + sp(2 banks x2) + oA/oB(1 bank each) = 8
            pst = ctx.enter_context(tc.tile_pool(name="pst", bufs=2, space="PSUM"))
            pss = ctx.enter_context(tc.tile_pool(name="pss", bufs=2, space="PSUM"))
            pso = ctx.enter_context(tc.tile_pool(name="pso", bufs=1, space="PSUM"))
            _ps_ctr = [0]

            def mk_ps():
                _ps_ctr[0] ^= 1
                t = "oA" if _ps_ctr[0] else "oB"
                return pso.tile([128, 512], f32, tag=t, name=f"ps_{t}")

            # ---------------- constants / weights into SBUF ----------------
            # Only what the LN1/QT path needs goes first on the sync queue;
            # bulk x loads ride the (otherwise idle) gpsimd queue.
            qkvT = wp.tile([128, 4, F3], b16, tag="qkvT")
            nc.sync.dma_start(out=qkvT, in_=qkvT_d[:, :].rearrange("(g p) f -> p g f", p=128))
            qb = wp.tile([128, 4], f32, tag="qb")
            nc.sync.dma_start(out=qb, in_=qb_d[:, :])
            ident = wp.tile([128, 128], b16, tag="ident")
            nc.sync.dma_start(out=ident, in_=ident_d[:, :])
            outTs = wp.tile([64, 8, C], b16, tag="outTs")
            outb = wp.tile([1, C], b16, tag="outb")
            fc1T = wp.tile([128, 4, HID], b16, tag="fc1T")
            fc1bp = wp.tile([128, 16], f32, tag="fc1bp")
            fc2b = wp.tile([1, C], b16, tag="fc2b")
            dww = wp.tile([128, 16, 9], f32, tag="dww")
            dwb = wp.tile([128, 16], f32, tag="dwb")
            maskb = wp.tile([128, EXT], b16, tag="maskb")

            def late_weight_dmas():
                nc.sync.dma_start(out=outTs, in_=outT_d[:, :].rearrange("(h p) f -> p h f", p=64))
                nc.sync.dma_start(out=outb, in_=outb_d[:, :])
                nc.sync.dma_start(out=fc1T, in_=fc1T_d[:, :].rearrange("(g p) f -> p g f", p=128))
                nc.sync.dma_start(out=fc1bp, in_=fc1bp_d[:, :])
                nc.sync.dma_start(out=fc2b, in_=fc2b_d[:, :])
                nc.sync.dma_start(out=dww, in_=dww_d[:, :].rearrange("(g p) t -> p g t", p=128))
                nc.sync.dma_start(out=dwb, in_=dwb_d[:].rearrange("(g p) -> p g", p=128))
                nc.sync.dma_start(
                    out=maskb,
                    in_=bass.AP(tensor=mask_d[:].tensor, offset=0, ap=[[0, 128], [1, EXT]]),
                )

            ones = wp.tile([1, C], b16, tag="ones")
            nc.vector.memset(ones, 1.0)
            onesq = wp.tile([128, 128], b16, tag="onesq")
            nc.vector.memset(onesq, 1.0)
            epsc = wp.tile([128, 1], f32, tag="epsc")
            nc.vector.memset(epsc, EPS)
            # touch Sqrt right away so its table load overlaps the first DMAs
            scr = wp.tile([1, 1], f32, tag="scr")
            nc.scalar.activation(out=scr, in_=epsc[0:1], func=AF.Sqrt)

            # ---------------- layer norm (token-major stats) ----------------
            def layer_norm_tile(xt, ts, lt, use_lnexp, act_apply=False):
                st = small.tile([128, 6], f32, tag="st")
                nc.vector.bn_stats(out=st[:ts], in_=xt[:ts])
                mv = small.tile([128, 2], f32, tag="mv")
                nc.vector.bn_aggr(out=mv[:ts], in_=st[:ts])
                if use_lnexp:
                    # rstd = exp(-0.5*ln(var+eps)); stays in the ln/exp table set
                    nc.scalar.activation(
                        out=mv[:ts, 1:2], in_=mv[:ts, 1:2], func=AF.Ln,
                        bias=epsc[:ts], scale=1.0,
                    )
                    nc.scalar.activation(
                        out=mv[:ts, 1:2], in_=mv[:ts, 1:2], func=AF.Exp, scale=-0.5,
                    )
                else:
                    nc.scalar.activation(
                        out=mv[:ts, 1:2], in_=mv[:ts, 1:2], func=AF.Sqrt,
                        bias=epsc[:ts], scale=1.0,
                    )
                    nc.vector.reciprocal(out=mv[:ts, 1:2], in_=mv[:ts, 1:2])
                if act_apply:
                    # (x-mu)*rstd on ACT as rstd*x + (-mu*rstd); Identity is
                    # in every table set
                    nmr = small.tile([128, 1], f32, tag="nmr")
                    nc.vector.scalar_tensor_tensor(
                        out=nmr[:ts], in0=mv[:ts, 0:1], scalar=-1.0,
                        in1=mv[:ts, 1:2], op0=OP.mult, op1=OP.mult,
                    )
                    nc.scalar.activation(
                        out=lt[:ts], in_=xt[:ts], func=AF.Identity,
                        bias=nmr[:ts], scale=mv[:ts, 1:2],
                    )
                else:
                    nc.vector.tensor_scalar(
                        out=lt[:ts], in0=xt[:ts],
                        scalar1=mv[:ts, 0:1], scalar2=mv[:ts, 1:2],
                        op0=OP.subtract, op1=OP.mult,
                    )

            def pe_transpose(lt, ts, dst4):
                # 4 chunk transposes (matmul by identity) into one psum bank,
                # then a single ACT cast into the c-major [128, 4, ts] slice
                # (ACT has slack in the LN and out-proj phases; Copy is in
                # every activation table set so no table swap).
                tp = pst.tile([128, 512], f32, tag="tr")
                for cc in range(4):
                    nc.tensor.matmul(
                        tp[:, cc * 128 : cc * 128 + ts],
                        lt[:ts, cc * 128 : (cc + 1) * 128],
                        ident[:ts, :ts],
                        start=True, stop=True,
                    )
                nc.scalar.activation(
                    out=dst4,
                    in_=tp.rearrange("p (c t) -> p c t", t=128)[:, :, :ts],
                    func=AF.Copy,
                )

            def layer_norm_T(src_d, tiles, dstT, use_lnexp=False):
                for i, (t0, ts) in enumerate(tiles):
                    xt = stage.tile([128, C], f32, tag="xf")
                    nc.gpsimd.dma_start(out=xt[:ts], in_=src_d[t0 : t0 + ts, :])
                    lt = stage.tile([128, C], b16, tag="xl")
                    layer_norm_tile(xt, ts, lt, use_lnexp, act_apply=(i % 2 == 1))
                    pe_transpose(lt, ts, dstT[:, :, t0 : t0 + ts])

            ln1x4 = big.tile([128, 4, NB], b16, tag="lx", name="ln1x4")
            ln1eT = big.tile([128, 4, EXT], b16, tag="le")

            # ext LN first so QT (and with it the exp stream) starts early
            layer_norm_T(xe_d, etiles, ln1eT)
            layer_norm_T(xb_d, btiles, ln1x4)
            late_weight_dmas()

            # ---------------- projections: QT, KT ----------------
            QT = big.tile([128, 4, EXT], b16, tag="qt")
            for f in range(4):
                for qc in range(2):
                    q0 = qc * QCH
                    ps = mk_ps()
                    for c in range(4):
                        nc.tensor.matmul(
                            ps[:, :QCH],
                            qkvT[:, c, f * 128 : (f + 1) * 128],
                            ln1eT[:, c, q0 : q0 + QCH],
                            start=(c == 0), stop=(c == 3),
                        )
                    # fold the q bias in on the way out (k bias is dropped --
                    # softmax-invariant; v bias folded into outb on host)
                    nc.vector.tensor_scalar_add(
                        QT[:, f, q0 : q0 + QCH], ps[:, :QCH], qb[:, f : f + 1]
                    )

            KT = [big.tile([128, NB], b16, tag=f"kt{c}", name=f"KT{c}") for c in range(4)]

            def emit_kt_chunk(f, ci):
                # uses the pst/"tr" psum tag: safe to weave between attention
                # groups whose oA/oB accumulators are live in the pso pool
                t0, tn = bchunks[ci]
                ps = pst.tile([128, 512], f32, tag="tr", name="ktps")
                for c in range(4):
                    nc.tensor.matmul(
                        ps[:, :tn],
                        qkvT[:, c, C + f * 128 : C + (f + 1) * 128],
                        ln1x4[:, c, t0 : t0 + tn],
                        start=(c == 0), stop=(c == 3),
                    )
                nc.vector.tensor_copy(out=KT[f][:, t0 : t0 + tn], in_=ps[:, :tn])

            for ci in range(len(bchunks)):
                emit_kt_chunk(0, ci)

            V5 = big.tile([128, len(btiles), 8, 65], b16, tag="v5")
            nc.vector.memset(V5[:, :, :, 64:65], 1.0)

            def emit_v5(i):
                t0, ts = btiles[i]
                ps = pst.tile([128, 512], f32, tag="tr", name="v5ps")
                for c in range(4):
                    nc.tensor.matmul(
                        ps[:ts],
                        ln1x4[:, c, t0 : t0 + ts],
                        qkvT[:, c, 2 * C : 3 * C],
                        start=(c == 0), stop=(c == 3),
                    )
                nc.vector.tensor_copy(
                    out=V5[:ts, i, :, 0:64],
                    in_=ps[:ts].rearrange("p (h d) -> p h d", d=64),
                )

            # ---------------- attention ----------------
            oTs = big.tile([64, 8, EXT], b16, tag="oTs")
            srow = big.tile([65, 8, QCH], b16, tag="srow")

            def attn_group(qc, pr, with_v5=False, feed_kt=None):
                q0 = qc * QCH
                hA, hB = 2 * pr, 2 * pr + 1
                oA = pso.tile([65, QCH], f32, tag="oA")
                oB = pso.tile([65, QCH], f32, tag="oB")
                for kt, (k0, kn) in enumerate(btiles):
                    if with_v5:
                        emit_v5(kt)
                    if feed_kt is not None and kt % 4 == 0:
                        emit_kt_chunk(feed_kt, kt // 4)
                    sp = pss.tile([128, 1024], f32, tag="sp")
                    nc.tensor.matmul(
                        sp[:kn, 0:QCH], KT[pr][0:64, k0 : k0 + kn],
                        QT[0:64, pr, q0 : q0 + QCH], start=True, stop=True,
                        tile_position=(0, 0),
                    )
                    nc.tensor.matmul(
                        sp[:kn, 512 : 512 + QCH], KT[pr][64:128, k0 : k0 + kn],
                        QT[64:128, pr, q0 : q0 + QCH], start=True, stop=True,
                        tile_position=(64, 0),
                    )
                    ex = atp.tile([128, 2, QCH], b16, tag="ex")
                    nc.scalar.activation(
                        out=ex[:kn],
                        in_=sp.rearrange("p (s x) -> p s x", x=512)[:kn, :, 0:QCH],
                        func=AF.Exp,
                    )
                    nc.tensor.matmul(
                        oA, V5[:kn, kt, hA, :], ex[:kn, 0, :],
                        start=(kt == 0), stop=(kt == len(btiles) - 1),
                    )
                    nc.tensor.matmul(
                        oB, V5[:kn, kt, hB, :], ex[:kn, 1, :],
                        start=(kt == 0), stop=(kt == len(btiles) - 1),
                    )
                # stash unnormalized o and the exp-sums (partition 64)
                nc.vector.tensor_copy(out=oTs[:, hA, q0 : q0 + QCH], in_=oA[0:64])
                nc.vector.tensor_copy(out=oTs[:, hB, q0 : q0 + QCH], in_=oB[0:64])
                nc.vector.tensor_copy(out=srow[64:65, hA, :], in_=oA[64:65])
                nc.vector.tensor_copy(out=srow[64:65, hB, :], in_=oB[64:65])

            def attn_norm(qc):
                q0 = qc * QCH
                # reciprocal of all 8 sums at once: 1/s = exp(-ln(s)), in place
                nc.scalar.activation(
                    out=srow[64:65].rearrange("p a b -> p (a b)"),
                    in_=srow[64:65].rearrange("p a b -> p (a b)"),
                    func=AF.Ln,
                )
                nc.scalar.activation(
                    out=srow[64:65].rearrange("p a b -> p (a b)"),
                    in_=srow[64:65].rearrange("p a b -> p (a b)"),
                    func=AF.Exp, scale=-1.0,
                )
                # broadcast 1/s to 64 partitions and normalize oTs
                for h in range(8):
                    rb = pst.tile([128, 512], f32, tag="tr")
                    nc.tensor.matmul(
                        rb[0:64, :QCH], onesq[64:65, 0:64], srow[64:65, h, :],
                        start=True, stop=True,
                    )
                    nc.vector.scalar_tensor_tensor(
                        out=oTs[:, h, q0 : q0 + QCH],
                        in0=oTs[:, h, q0 : q0 + QCH],
                        scalar=1.0, in1=rb[0:64, :QCH],
                        op0=OP.bypass, op1=OP.mult,
                    )

            # qc0/pr0 carries the V5 projection; KT[f] lands just-in-time
            attn_group(0, 0, with_v5=True)
            attn_group(0, 1, feed_kt=1)
            attn_group(0, 2, feed_kt=2)
            attn_group(0, 3, feed_kt=3)
            attn_norm(0)
            for pr in range(4):
                attn_group(1, pr)
            attn_norm(1)

            # ---------------- out-proj + residual + LN2 ----------------
            a_sb = big.tile([128, 7, C], b16, tag="a_sb")
            ln2aT = big.tile([128, 4, EXT], b16, tag="le")  # reuse ln1eT slot

            for i, (t0, ts) in enumerate(etiles):
                ps = mk_ps()
                for h in range(8):
                    nc.tensor.matmul(
                        ps, oTs[:, h, t0 : t0 + ts], outTs[:, h, :],
                        start=(h == 0), stop=False,
                    )
                nc.tensor.matmul(ps, ones[:, :ts], outb, start=False, stop=True)
                xt = stage.tile([128, C], f32, tag="xf")
                nc.sync.dma_start(out=xt[:ts], in_=xe_d[t0 : t0 + ts, :])
                nc.vector.tensor_add(out=a_sb[:ts, i, :], in0=xt[:ts], in1=ps[:ts])
                lt = stage.tile([128, C], b16, tag="xl")
                layer_norm_tile(a_sb[:, i, :], ts, lt, use_lnexp=True)
                pe_transpose(lt, ts, ln2aT[:, :, t0 : t0 + ts])

            # ---------------- MLP: fc1 -> dwconv+mask -> gelu -> fc2 ----------------
            # fc2 weights arrive late, into the slots KT vacated after attention
            fc2Ta = big.tile([128, 8, C], b16, tag="kt0")
            nc.sync.dma_start(
                out=fc2Ta, in_=fc2T_d[0:1024, :].rearrange("(g p) f -> p g f", p=128)
            )
            fc2Tb = big.tile([128, 8, C], b16, tag="kt1")
            nc.sync.dma_start(
                out=fc2Tb, in_=fc2T_d[1024:2048, :].rearrange("(g p) f -> p g f", p=128)
            )
            ghT = big.tile([128, 16, OWN], b16, tag="lx")  # reuse ln1x4 slot
            SPAN = RPC * (WI + 2)          # 812 flat conv span (2 junk cols/row)
            HSP = SPAN // 2                # 406 = 7 rows x 58, per psum half-bank
            PADW = EXTR * (WI + 2) + 2     # 930: +2 so the last tap's junk reads stay in-bounds

            def mlp_fc1(g):
                # fc1 + masked scatter (fc1 bias folded in) into a fresh padded
                # conv buffer; returns the pad tile for the conv stage.
                pad = padp.tile([128, PADW], b16, tag="pad", name="pad")
                padv = pad[:, : PADW - 2].rearrange("p (r x) -> p r x", x=WI + 2)
                nc.vector.memset(pad[:, PADW - 2 :], 0.0)
                nc.vector.memset(padv[:, :, 0:1], 0.0)
                nc.vector.memset(padv[:, :, WI + 1 : WI + 2], 0.0)
                for qc in range(2):
                    q0 = qc * QCH
                    ps = mk_ps()
                    for c in range(4):
                        nc.tensor.matmul(
                            ps[:, :QCH],
                            fc1T[:, c, g * 128 : (g + 1) * 128],
                            ln2aT[:, c, q0 : q0 + QCH],
                            start=(c == 0), stop=(c == 3),
                        )
                    nc.vector.scalar_tensor_tensor(
                        out=padv[:, qc * 8 : (qc + 1) * 8, 1 : WI + 1],
                        in0=ps[:, :QCH].rearrange("p (r x) -> p r x", x=WI),
                        scalar=fc1bp[:, g : g + 1],
                        in1=maskb[:, q0 : q0 + QCH].rearrange("p (r x) -> p r x", x=WI),
                        op0=OP.add, op1=OP.mult,
                    )
                return pad

            def mlp_conv(g, pad):
                # 3x3 depthwise conv: 7 taps on PE (diagonal stationary, psum
                # accumulate over two half-spans), DVE_TAPS[0] in-place on psum,
                # DVE_TAPS[1] compacts (junk cols dropped) into ghT, then gelu
                # in place on ghT.
                dgt = dgp.tile([128, len(PE_TAPS), 128], b16, tag="dg")
                nc.sync.dma_start(
                    out=dgt,
                    in_=dwdiag_d[g].rearrange("p (t c) -> p t c", c=128),
                )
                cps = pss.tile([128, 1024], f32, tag="sp")
                for s in range(2):
                    for j, tap in enumerate(PE_TAPS):
                        dy, dx = tap // 3, tap % 3
                        off = dy * (WI + 2) + dx + s * HSP
                        nc.tensor.matmul(
                            cps[:, s * 512 : s * 512 + HSP],
                            dgt[:, j, :],
                            pad[:, off : off + HSP],
                            start=(j == 0), stop=(j == len(PE_TAPS) - 1),
                        )
                cps3 = cps.rearrange("p (s x) -> p s x", x=512)[:, :, 0:HSP]
                tap = DVE_TAPS[0]
                off = (tap // 3) * (WI + 2) + tap % 3
                nc.vector.scalar_tensor_tensor(
                    out=cps3,
                    in0=pad[:, off : off + SPAN].rearrange("p (s x) -> p s x", x=HSP),
                    scalar=dww[:, g, tap : tap + 1],
                    in1=cps3, op0=OP.mult, op1=OP.add,
                )
                tap = DVE_TAPS[1]
                off = (tap // 3) * (WI + 2) + tap % 3
                for s in range(2):
                    nc.vector.scalar_tensor_tensor(
                        out=ghT[:, g, s * (OWN // 2) :][:, : OWN // 2].rearrange(
                            "p (r x) -> p r x", x=WI
                        ),
                        in0=pad[:, off + s * HSP :][:, :HSP].rearrange(
                            "p (r x) -> p r x", x=WI + 2
                        )[:, :, 0:WI],
                        scalar=dww[:, g, tap : tap + 1],
                        in1=cps.rearrange("p (s x) -> p s x", x=512)[
                            :, s, :HSP
                        ].rearrange("p (r x) -> p r x", x=WI + 2)[:, :, 0:WI],
                        op0=OP.mult, op1=OP.add,
                    )
                nc.scalar.activation(
                    out=ghT[:, g, :], in_=ghT[:, g, :],
                    func=AF.Gelu, bias=dwb[:, g : g + 1], scale=1.0,
                )

            # software-pipelined with one-group skew so the DVE scatter of
            # g+1 overlaps the PE conv taps of g
            prev = (0, mlp_fc1(0))
            for g in range(1, 16):
                pad = mlp_fc1(g)
                mlp_conv(*prev)
                prev = (g, pad)
            mlp_conv(*prev)

            # ---------------- fc2 + final residual ----------------
            for i, (t0, ts) in enumerate(otiles):
                ps = mk_ps()
                for k in range(16):
                    f2 = fc2Ta[:, k, :] if k < 8 else fc2Tb[:, k - 8, :]
                    nc.tensor.matmul(
                        ps[:ts],
                        ghT[:, k, t0 : t0 + ts],
                        f2,
                        start=(k == 0), stop=False,
                    )
                nc.tensor.matmul(ps[:ts], ones[:, :ts], fc2b, start=False, stop=True)
                at = stage.tile([128, C], b16, tag="xf")
                n1 = min(ts, 128 - WI)  # rows from a tile i (partitions WI..)
                nc.sync.dma_start(out=at[:n1], in_=a_sb[WI : WI + n1, i, :])
                if ts > n1:
                    nc.sync.dma_start(
                        out=at[n1:ts], in_=a_sb[0 : ts - n1, i + 1, :]
                    )
                ot = stage.tile([128, C], f32, tag="xa")
                nc.vector.tensor_add(out=ot[:ts], in0=at[:ts], in1=ps[:ts])
                nc.sync.dma_start(out=out_d[t0 : t0 + ts, :], in_=ot[:ts])

    return nc


def _prep_host(inputs):
    import ml_dtypes

    bf16 = ml_dtypes.bfloat16
    f32 = np.float32

    g = {k: np.asarray(v) for k, v in inputs.items()}
    x = g["x"].astype(f32)
    ln1_w, ln1_b = g["ln1_w"].astype(f32), g["ln1_b"].astype(f32)
    ln2_w, ln2_b = g["ln2_w"].astype(f32), g["ln2_b"].astype(f32)
    qkv_w, qkv_b = g["qkv_w"].astype(f32), g["qkv_b"].astype(f32)
    out_w, out_b = g["out_w"].astype(f32), g["out_b"].astype(f32)
    fc1_w, fc1_b = g["fc1_w"].astype(f32), g["fc1_b"].astype(f32)
    fc2_w, fc2_b = g["fc2_w"].astype(f32), g["fc2_b"].astype(f32)
    dw_w, dw_b = g["dw_w"].astype(f32), g["dw_b"].astype(f32)
    temp = float(np.asarray(g["temperature"]))

    # fold LN affine into the following matmul; fold 1/temperature into W_q
    qkv_w2 = qkv_w * ln1_w[None, :]
    qkv_b2 = qkv_b + qkv_w @ ln1_b
    qkv_w2[:C] /= temp
    qkv_b2[:C] /= temp
    fc1_w2 = fc1_w * ln2_w[None, :]
    fc1_b2 = fc1_b + fc1_w @ ln2_b
    # v bias shifts attention output by a constant -> fold into out_b;
    # k bias is softmax-invariant -> dropped entirely.
    out_b2 = out_b + out_w @ qkv_b2[2 * C :]

    dwf = dw_w.reshape(HID, 9)
    dwdiag = np.zeros((16, 128, len(PE_TAPS), 128), f32)
    for gi in range(16):
        for j, tap in enumerate(PE_TAPS):
            dwdiag[gi, np.arange(128), j, np.arange(128)] = dwf[
                gi * 128 : (gi + 1) * 128, tap
            ]

    shared = {
        "qkvT": np.ascontiguousarray(qkv_w2.T).astype(bf16),
        "qb": np.ascontiguousarray(qkv_b2[:C].reshape(4, 128).T).astype(f32),
        "outT": np.ascontiguousarray(out_w.T).astype(bf16),
        "outb": out_b2[None, :].astype(bf16),
        "fc1T": np.ascontiguousarray(fc1_w2.T).astype(bf16),
        "fc1bp": np.ascontiguousarray(fc1_b2.reshape(16, 128).T).astype(f32),
        "fc2T": np.ascontiguousarray(fc2_w.T).astype(bf16),
        "fc2b": fc2_b[None, :].astype(bf16),
        "dww": np.ascontiguousarray(dwf).astype(f32),
        "dwb": dw_b.astype(f32),
        "dwdiag": np.ascontiguousarray(dwdiag.reshape(16, 128, -1)).astype(bf16),
        "ident": np.eye(128, dtype=f32).astype(bf16),
    }

    ximg = x.reshape(B, HI, WI, C)
    in_maps = []
    for c in range(NCORES):
        b, qi = c // 4, c % 4
        r0 = RPC * qi
        xe = np.zeros((EXTR, WI, C), f32)
        mask = np.zeros((EXTR, WI), f32)
        for e in range(EXTR):
            r = r0 - 1 + e
            if 0 <= r < HI:
                xe[e] = ximg[b, r]
                mask[e] = 1.0
        m = dict(shared)
        m["xb"] = np.ascontiguousarray(x[b])
        m["xe"] = np.ascontiguousarray(xe.reshape(EXT, C))
        m["mask"] = mask.reshape(EXT).astype(bf16)
        in_maps.append(m)
    return in_maps


def _run(inputs, trace=False):
    from concourse.bass_utils import run_bass_kernel_spmd

    if "nc" not in _CACHE:
        nc = _build_nc()
        nc.finalize()
        _CACHE["nc"] = nc
    nc = _CACHE["nc"]
    in_maps = _prep_host(inputs)
    res = run_bass_kernel_spmd(nc, in_maps, core_ids=list(range(NCORES)), trace=trace)

    x = np.asarray(inputs["x"])
    out = np.zeros((B, NB, C), np.float32)
    for c in range(NCORES):
        b, qi = c // 4, c % 4
        r0 = RPC * qi
        out[b, r0 * WI : (r0 + RPC) * WI, :] = res.results[c]["out"]
    return out.astype(x.dtype, copy=False), res


def kernel(**inputs) -> np.ndarray:
    out, _ = _run(inputs, trace=False)
    return out
